# revision 42
# baseline (speedup 1.0000x reference)
"""Tensor-parallel Llama attention for 8 TRN2 NeuronCores.

Sharding: core d handles batch d//4 and q-head group g = d%4 (q heads
4g..4g+3, kv head g — GQA group-aligned so each core needs exactly one
kv head).  Wq/Wk/Wv are row-sharded, Wo column-sharded; the per-batch
partial o_proj outputs of 4 cores are summed on the host.

Device layouts (prepared host-side, bf16):
  hsT  [16,128,S]   hidden_states[b].T, HID on partitions in 16 chunks
  wqT  [16,128,512] Wq_shard.T          wkT/wvT [16,128,128]
  woT  [4,128,2048] Wo_shard.T (4 contraction chunks of the 512 local dims)
  cosT/sinT [128,S] RoPE tables in [head_dim, seq] layout
  maskb [4,128,512] 0/1 causal masks for the 4 diagonal-block phases

Performance structure (v2): the tensor engine p-state ramps to full
clock only under continuous execution, so the whole kernel is scheduled
as one dense PE instruction stream:
  - scores/PV stay in transposed layout; softmax sums accumulate on the
    vector engine (scalar_tensor_tensor, 2x SBUF mode) instead of
    per-block ones-matmuls; a single f32r fold matmul + f32r broadcast
    matmul per (gi, head) recovers the denominators.
  - causal masking is a post-exp multiplicative 0/1 mask on the
    (width-trimmed) diagonal blocks, off the score-matmul critical path.
  - Q-projection chains and o_proj tiles are interleaved one matmul at
    a time between attention blocks so the PE never idles while the
    activation engine computes exp.
  - o_proj staging copies run on the scalar engine; softmax
    normalization is deferred one head so its fold matmul never stalls
    the PE; outputs are written bf16; input DMAs are split across the
    SP and Activation hardware DGE queues.
"""

import sys

sys.path.insert(0, "/opt/trn_rl_repo")

import numpy as np
import ml_dtypes

B, S, HID = 2, 2048, 2048
NH, NKV, HD = 16, 4, 128
THETA = 10000.0
NCORES = 8
HPC = 4            # q heads per core
QDIM = HPC * HD    # 512 local q dims
KT = HID // 128    # 16 contraction chunks
SB = S // 512      # 4 column groups of 512
ST = S // 128      # 16 row tiles of 128

_CACHE = {}


def _patch_tile_drain():
    """This walrus build caps sync waits per CTRL instruction below what the
    stock Tile kernel-tail drain carries; split them into single-wait NOPs."""
    import bass_rust
    import concourse.tile as tile
    from concourse.tile import ScopedClock

    if getattr(tile.TileContext, "_drain_split_patched", False):
        return

    def _split_drain_and_barrier(self, tick_clock, wait_clock):
        ticks = list(tick_clock.global_clock)
        for i, v in enumerate(ticks):
            if v > 0:
                single = [0] * len(ticks)
                single[i] = v
                nop = self.nc.sync.nop(nofuse=True, hint=f"drain_wait_{i}")
                wait_clock.add_sem_waits(
                    nop.ins, ScopedClock({None: bass_rust.VectorClock(single)})
                )
        self.nc.sync.drain()
        self.nc.all_engine_barrier()
        assert self.sems is not None
        popped = self.nc._tile_sem_poison_stack.pop()
        assert popped is self._sem_poison
        self.nc.clear_and_free_semaphores(list(self.sems.allocated().values()))
        self.nc.all_engine_barrier()

    tile.TileContext._drain_and_barrier = _split_drain_and_barrier
    tile.TileContext._drain_split_patched = True


def _legalize_waits(nc, max_waits=1):
    """This walrus build rejects instructions carrying more than ~2 sync
    waits.  Hoist the excess onto single-wait NOPs inserted just before the
    instruction in its block (same engine => same instruction stream, so
    the waits still complete before the op issues)."""
    import concourse.mybir as mybir

    n_split = 0
    for block in nc.m.functions[0].blocks:
        insts = list(block.instructions)
        out = []
        for inst in insts:
            si = getattr(inst, "sync_info", None)
            if si is not None and si.on_wait and len(si.on_wait) > max_waits:
                waits = list(si.on_wait)
                keep = waits[:max_waits]
                for j, w in enumerate(waits[max_waits:]):
                    out.append(
                        mybir.InstNoOp(
                            name=f"{inst.name}_hw{j}",
                            engine=inst.engine,
                            bass_nofuse=True,
                            sync_info=mybir.SyncInfo(on_wait=[w], on_update=[]),
                        )
                    )
                si.on_wait = keep
                n_split += 1
            out.append(inst)
        block.instructions = out
    return n_split


def _build_nc():
    import concourse.bass as bass
    import concourse.mybir as mybir
    import concourse.tile as tile
    from concourse.masks import make_identity

    _patch_tile_drain()

    bf = mybir.dt.bfloat16
    f32 = mybir.dt.float32
    f32r = mybir.dt.float32r
    Exp = mybir.ActivationFunctionType.Exp
    Ln = mybir.ActivationFunctionType.Ln
    MUL = mybir.AluOpType.mult
    ADD = mybir.AluOpType.add

    nc = bass.Bass()
    hsT = nc.declare_dram_parameter("hsT", [KT, 128, S], bf, isOutput=False)
    wqT = nc.declare_dram_parameter("wqT", [128, KT * QDIM], bf, isOutput=False)
    wkT = nc.declare_dram_parameter("wkT", [128, KT * HD], bf, isOutput=False)
    wvT = nc.declare_dram_parameter("wvT", [128, KT * HD], bf, isOutput=False)
    woT = nc.declare_dram_parameter("woT", [128, 4 * HID], bf, isOutput=False)
    cosT = nc.declare_dram_parameter("cosT", [128, S], bf, isOutput=False)
    sinT = nc.declare_dram_parameter("sinT", [128, S], bf, isOutput=False)
    maskb = nc.declare_dram_parameter("maskb", [128, 4 * 512], bf, isOutput=False)
    identD = nc.declare_dram_parameter("identD", [128, 128], bf, isOutput=False)
    out = nc.declare_dram_parameter("out", [S, HID], bf, isOutput=True)

    inv_sqrt_d = 1.0 / float(np.sqrt(HD))

    with tile.TileContext(nc) as tc:
        with (
            tc.tile_pool(name="resid", bufs=1) as resid,
            tc.tile_pool(name="probs", bufs=6) as probs_pool,
            tc.tile_pool(name="accp", bufs=2) as acc_pool,
            tc.tile_pool(name="rcp", bufs=2) as rc_pool,
            tc.tile_pool(name="ropes", bufs=2) as rope_pool,
            tc.tile_pool(name="ostage", bufs=4) as ostage_pool,
            tc.tile_pool(name="ps", bufs=1, space="PSUM") as ps,
        ):
            # PSUM bank budget (8 banks total):
            #   score 2 + qfill 1 + pv 2 + oproj 2 + small 1 == 8
            def ps_tile(tag, bufs, shape=(128, 512), dtype=f32):
                t = ps.tile(list(shape), dtype, tag=tag, bufs=bufs,
                            name=f"ps_{tag}")
                return t

            hs_sb = resid.tile([128, KT * S], bf)
            wq_sb = resid.tile([128, KT * QDIM], bf)
            wk_sb = resid.tile([128, KT * HD], bf)
            wv_sb = resid.tile([128, KT * HD], bf)
            wo_sb = resid.tile([128, 4 * HID], bf)
            cos_sb = resid.tile([128, S], bf)
            sin_sb = resid.tile([128, S], bf)
            maskb_sb = resid.tile([128, 4 * 512], bf)
            ones_bf = resid.tile([128, 1], bf)
            onesr_bf = resid.tile([1, 128], bf)
            ident = resid.tile([128, 128], bf)
            qT_sb = resid.tile([128, HPC * S], bf)
            kT_sb = resid.tile([128, S], bf)
            vT_sb = resid.tile([128, S], bf)
            vn_sb = resid.tile([128, S], bf)
            at_sb = resid.tile([128, HPC * S], bf)
            opart_sb = resid.tile([128, 16 * 512], bf)

            # ---- input DMAs, split across the two HW DGE queues ----
            # Weights are pre-reshaped host-side to the SBUF layout so each
            # is ONE large-row DMA.  hs chunks alternate queues so arrival
            # tracks the per-chunk k+v projection consumption order.
            # SP queue: wk, wv, ident, even hs chunks, wo.
            # ACT queue: cos, sin, maskb, odd hs chunks, wq.
            # The ACT queue must drain its dma_start issues before the first
            # exp can run (in-order engine), so it only carries small early
            # tensors plus the last 4 hs chunks.
            def hs_dma(eng, kk):
                eng.dma_start(hs_sb[:, kk * S:(kk + 1) * S], hsT[kk])

            nc.sync.dma_start(wk_sb[:], wkT[:])
            hs_dma(nc.sync, 0)
            nc.sync.dma_start(wv_sb[:], wvT[:])
            hs_dma(nc.sync, 1)
            nc.sync.dma_start(ident[:], identD[:])
            for kk in range(2, 12):
                hs_dma(nc.sync, kk)
            nc.sync.dma_start(wq_sb[:], wqT[:])
            nc.sync.dma_start(wo_sb[:], woT[:])
            nc.scalar.dma_start(cos_sb[:], cosT[:])
            nc.scalar.dma_start(sin_sb[:], sinT[:])
            nc.scalar.dma_start(maskb_sb[:], maskb[:])
            for kk in range(12, KT):
                hs_dma(nc.scalar, kk)

            nc.vector.memset(ones_bf[:], 1.0)
            nc.vector.memset(onesr_bf[:], 1.0)

            # ---- RoPE on a [d, s-window] slice (in place) ----
            # dst_lo: column offset in the destination tile; s_lo: the
            # s-window it corresponds to (for the cos/sin tables).
            # eng=gpsimd keeps filler RoPEs off the loaded vector engine.
            def rope(h, dst_lo, s_lo, width=512, eng=None):
                e = eng if eng is not None else nc.vector
                dl = slice(dst_lo, dst_lo + width)
                sl = slice(s_lo, s_lo + width)
                rot = rope_pool.tile([128, 512], bf, tag="rot", name="rot")
                e.tensor_scalar_mul(rot[0:64, 0:width], h[64:128, dl], -1.0)
                e.tensor_copy(rot[64:128, 0:width], h[0:64, dl])
                e.tensor_mul(h[:, dl], h[:, dl], cos_sb[:, sl])
                e.tensor_mul(rot[:, 0:width], rot[:, 0:width], sin_sb[:, sl])
                e.tensor_add(h[:, dl], h[:, dl], rot[:, 0:width])

            # ---- K + V(sg0,sg1) projection, kk-outer: 6 matmuls per hs
            # chunk keep the PE fed at the DMA chunk-arrival rate ----
            ktiles = [ps_tile("score", 2), ps_tile("score", 2),
                      ps_tile("qfill", 1), ps_tile("small", 1)]
            vtiles = [ps_tile("pv", 2), ps_tile("pv", 2)]
            for kk in range(KT):
                for sg in range(4):
                    nc.tensor.matmul(
                        ktiles[sg][:],
                        wk_sb[:, kk * HD:(kk + 1) * HD],
                        hs_sb[:, kk * S + sg * 512: kk * S + sg * 512 + 512],
                        start=(kk == 0), stop=(kk == KT - 1),
                    )
                for sg in range(2):
                    nc.tensor.matmul(
                        vtiles[sg][:],
                        wv_sb[:, kk * HD:(kk + 1) * HD],
                        hs_sb[:, kk * S + sg * 512: kk * S + sg * 512 + 512],
                        start=(kk == 0), stop=(kk == KT - 1),
                    )
            for sg in range(4):
                nc.vector.tensor_copy(
                    kT_sb[:, sg * 512:(sg + 1) * 512], ktiles[sg][:])
            rope(kT_sb, 0, 0)
            for sg in range(2):
                nc.vector.tensor_copy(
                    vT_sb[:, sg * 512:(sg + 1) * 512], vtiles[sg][:])

            # ---- generic 16-matmul projection chain -> dst slice ----
            # Yields micro-ops so chains can be interleaved as PE fillers.
            # m: 128-row block of W; s_lo: s-window; dst_lo: column offset
            # in dst where the [128, 512] result lands.
            def proj_chain_ops(w_sb, wdim, m, dst, dst_lo, s_lo, tag,
                               rope_after, rope_eng=None):
                def ops():
                    t = ps_tile(tag, {"qfill": 1}.get(tag, 2))
                    for kk in range(KT):
                        yield lambda kk=kk, t=t: nc.tensor.matmul(
                            t[:],
                            w_sb[:, kk * wdim + m * 128: kk * wdim + (m + 1) * 128],
                            hs_sb[:, kk * S + s_lo: kk * S + s_lo + 512],
                            start=(kk == 0), stop=(kk == KT - 1),
                        )

                    def fin(t=t):
                        nc.vector.tensor_copy(dst[:, dst_lo:dst_lo + 512],
                                              t[:])
                        if rope_after:
                            rope(dst, dst_lo, s_lo, eng=rope_eng)
                    yield fin
                return ops()

            # ---- V transpose micro-ops for one s-group (PE + DVE copy) ----
            def vtrans_ops(sg):
                def ops():
                    for tj in range(4 * sg, 4 * sg + 4):
                        tp = ps_tile("oproj", 2, (128, 128), bf)

                        def one(tj=tj, tp=tp):
                            nc.tensor.transpose(
                                tp[:], vT_sb[:, tj * 128:(tj + 1) * 128],
                                ident[:])
                            nc.vector.tensor_copy(
                                vn_sb[:, tj * 128:(tj + 1) * 128], tp[:])
                        yield one
                return ops()

            # ---- o_proj micro-ops for one (st, eg) output tile ----
            def oproj_ops(st, eg):
                def ops():
                    t = ps_tile("oproj", 2)
                    for h in range(HPC):
                        yield lambda h=h, t=t: nc.tensor.matmul(
                            t[:],
                            at_sb[:, h * S + st * 128: h * S + st * 128 + 128],
                            wo_sb[:, h * HID + eg * 512: h * HID + eg * 512 + 512],
                            start=(h == 0), stop=(h == HPC - 1),
                        )

                    def fin(t=t):
                        ostage = ostage_pool.tile([128, 512], bf,
                                                  name="ostage")
                        if (st + eg) % 2 == 0:
                            nc.scalar.copy(ostage[:], t[:])
                        else:
                            nc.vector.tensor_copy(ostage[:], t[:])
                        nc.sync.dma_start(
                            out[st * 128:(st + 1) * 128,
                                eg * 512:(eg + 1) * 512],
                            ostage[:])
                    yield fin
                return ops()

            # ---- split o_proj for the last i-group: heads 0..2 are
            # accumulated and spilled (bf16) during gi=3's attention; the
            # tail only runs an identity-restore matmul + the h3 matmul ----
            def oproj_part_a(st, eg, idx):
                def ops():
                    t = ps_tile("oproj", 2)
                    for h in range(3):
                        yield lambda h=h, t=t: nc.tensor.matmul(
                            t[:],
                            at_sb[:, h * S + st * 128: h * S + st * 128 + 128],
                            wo_sb[:, h * HID + eg * 512: h * HID + eg * 512 + 512],
                            start=(h == 0), stop=(h == 2),
                        )

                    def spill(t=t):
                        nc.vector.tensor_copy(
                            opart_sb[:, idx * 512:(idx + 1) * 512], t[:])
                    yield spill
                return ops()

            def oproj_part_b(st, eg, idx):
                def ops():
                    t = ps_tile("oproj", 2)
                    yield lambda t=t: nc.tensor.matmul(
                        t[:], ident[:],
                        opart_sb[:, idx * 512:(idx + 1) * 512],
                        start=True, stop=False, skip_group_check=True,
                    )
                    yield lambda t=t: nc.tensor.matmul(
                        t[:],
                        at_sb[:, 3 * S + st * 128: 3 * S + st * 128 + 128],
                        wo_sb[:, 3 * HID + eg * 512: 3 * HID + eg * 512 + 512],
                        start=False, stop=True, skip_group_check=True,
                    )

                    def fin(t=t):
                        ostage = ostage_pool.tile([128, 512], bf,
                                                  name="ostage")
                        if (st + eg) % 2 == 0:
                            nc.scalar.copy(ostage[:], t[:])
                        else:
                            nc.vector.tensor_copy(ostage[:], t[:])
                        nc.sync.dma_start(
                            out[st * 128:(st + 1) * 128,
                                eg * 512:(eg + 1) * 512],
                            ostage[:])
                    yield fin
                return ops()

            # ---- filler machinery: a queue of PE micro-op generators ----
            # fills[i] = (key, generator, op_count); popping runs one op.
            fills = []
            done_keys = set()
            state = {"ops": 0}

            def add_fill(key, gen, n_ops):
                fills.append((key, gen))
                state["ops"] += n_ops

            def pop_fill(n=1):
                k = 0
                while fills and k < n:
                    key, gen = fills[0]
                    try:
                        next(gen)()
                        k += 1
                        state["ops"] -= 1
                    except StopIteration:
                        done_keys.add(key)
                        fills.pop(0)
                return k

            def drain_until(key):
                while key not in done_keys:
                    if not pop_fill(1):
                        raise RuntimeError(f"filler {key} was never queued")

            def drain_all():
                while fills:
                    pop_fill(4)

            # Phase 1 tail: transpose v sg0/sg1, q head 0 / s-group 0,
            # plus RoPE for the remaining k s-groups (DVE-only, overlaps).
            for op in vtrans_ops(0):
                op()
            for op in proj_chain_ops(wq_sb, QDIM, 0, qT_sb, 0, 0, "qfill",
                                     True):
                op()
            for sg in range(1, 4):
                rope(kT_sb, sg * 512, sg * 512)
            for op in vtrans_ops(1):
                op()

            # Filler queue: v s-groups 2..3 (+transposes), then q chains.
            for sg in range(2, 4):
                add_fill(f"v{sg}", proj_chain_ops(
                    wv_sb, HD, 0, vT_sb, sg * 512, sg * 512, "qfill",
                    False), 17)
                add_fill(f"vt{sg}", vtrans_ops(sg), 4)
            for h in range(1, HPC):
                add_fill(f"q{h}g0", proj_chain_ops(
                    wq_sb, QDIM, h, qT_sb, h * S, 0, "qfill", True), 17)
            for sg in range(1, 4):
                for h in range(HPC):
                    add_fill(f"q{h}g{sg}", proj_chain_ops(
                        wq_sb, QDIM, h, qT_sb, h * S + sg * 512, sg * 512,
                        "qfill", True), 17)
            done_keys.add("q0g0")
            done_keys.add("v0")
            done_keys.add("v1")
            done_keys.add("vt0")
            done_keys.add("vt1")

            # Deferred softmax-denominator tail for one (gi, h): issued a
            # few blocks into the NEXT head so the fold matmul never stalls
            # the in-order PE stream waiting on the DVE accumulator chain.
            def make_normalize(gi, h, pv, acc):
                def run():
                    fold = ps_tile("small", 1, (1, 512))
                    nc.tensor.matmul(
                        fold[:], ones_bf[:], acc[:], start=True, stop=True)
                    # 1/x as exp(-ln(x)) on the scalar engine: the DVE
                    # reciprocal is element-serial (~3.3us for 512 lanes on
                    # one partition) and would block the accumulator stream.
                    lnf = rc_pool.tile([1, 512], f32, tag="lnf", name="lnf")
                    nc.scalar.activation(lnf[:], fold[:], Ln)
                    rc1 = rc_pool.tile([1, 512], bf, name="rc1")
                    nc.scalar.activation(rc1[:], lnf[:], Exp, scale=-1.0)
                    bc = ps_tile("small", 1)
                    nc.tensor.matmul(
                        bc[:], onesr_bf[:], rc1[:], start=True, stop=True)
                    a_sl = at_sb[:, h * S + gi * 512:
                                 h * S + gi * 512 + 512]
                    nc.vector.tensor_copy(a_sl, pv[:])
                    nc.vector.tensor_mul(a_sl, a_sl, bc[:])
                    if gi == SB - 1 and h == HPC - 2:
                        # last i-group: pre-accumulate heads 0..2 of o_proj
                        # during the final head's attention
                        for i2, (st, eg) in enumerate(
                                (st, eg) for st in range(12, 16)
                                for eg in range(SB)):
                            add_fill(f"oa{st}e{eg}",
                                     oproj_part_a(st, eg, i2), 4)
                    if h == HPC - 1:
                        # at(gi) complete: o_proj(gi) becomes filler work
                        for i2, (st, eg) in enumerate(
                                (st, eg) for st in range(4 * gi, 4 * gi + 4)
                                for eg in range(SB)):
                            if gi == SB - 1:
                                add_fill(f"ob{st}e{eg}",
                                         oproj_part_b(st, eg, i2), 3)
                            else:
                                add_fill(f"o{st}e{eg}",
                                         oproj_ops(st, eg), 5)
                return run

            # ---- attention: gi-outer; o_proj(gi-1) drains as filler ----
            n_blocks_total = HPC * sum(4 * gi + 4 for gi in range(SB))
            blocks_left = [n_blocks_total]
            pending = [None]

            def pops_for_slot():
                # keep the filler backlog draining evenly across the
                # remaining attention blocks (2 PE matmuls per block).
                if blocks_left[0] <= 0:
                    return 2
                r = state["ops"] / blocks_left[0]
                return max(1, min(4, int(np.ceil(r)) - 1))

            for gi in range(SB):
                for sg in range(1, gi + 1):
                    drain_until(f"vt{sg}")
                ntj = 4 * gi + 4
                for h in range(HPC):
                    drain_until(f"q{h}g{gi}")
                    # prefetch the NEXT head's q chain now so its RoPE (DVE)
                    # has a whole head of slack before its first score matmul
                    if h < HPC - 1:
                        drain_until(f"q{h + 1}g{gi}")
                    elif gi < SB - 1:
                        drain_until(f"q0g{gi + 1}")
                    qh = qT_sb[:, h * S:(h + 1) * S]
                    pv = ps_tile("pv", 2)
                    acc = acc_pool.tile([128, 512], bf, name="acc")
                    for tj in range(ntj):
                        p = tj - 4 * gi  # >=0 on diagonal blocks
                        off = 128 * p if p > 0 else 0
                        sc = ps_tile("score", 2)
                        nc.tensor.matmul(
                            sc[:, off:512],
                            kT_sb[:, tj * 128:(tj + 1) * 128],
                            qh[:, gi * 512 + off: gi * 512 + 512],
                            start=True, stop=True,
                        )
                        pop_fill(1)
                        pb = probs_pool.tile([128, 512], bf, name="pb")
                        nc.scalar.activation(
                            pb[:, off:512], sc[:, off:512], Exp,
                            scale=inv_sqrt_d)
                        if p >= 0:
                            # causal: only the 128-wide triangle band of the
                            # diagonal block needs masking; columns beyond
                            # off+128 are fully valid
                            mw = min(off + 128, 512)
                            nc.vector.tensor_mul(
                                pb[:, off:mw], pb[:, off:mw],
                                maskb_sb[:, p * 512 + off:p * 512 + mw])
                        if tj == 0:
                            nc.vector.tensor_copy(acc[:], pb[:])
                        else:
                            nc.vector.tensor_add(
                                acc[:, off:512], acc[:, off:512],
                                pb[:, off:512])
                        nc.tensor.matmul(
                            pv[:, off:512],
                            vn_sb[:, tj * 128:(tj + 1) * 128],
                            pb[:, off:512],
                            start=(tj == 0), stop=(tj == ntj - 1),
                            skip_group_check=True,
                        )
                        blocks_left[0] -= 1
                        pop_fill(pops_for_slot())
                        if tj == 3 and pending[0] is not None:
                            pending[0]()
                            pending[0] = None
                    pending[0] = make_normalize(gi, h, pv, acc)
            pending[0]()
            drain_all()

    _legalize_waits(nc)
    return nc


def _host_prep(hidden_states, Wq, Wk, Wv, Wo, position_ids):
    bf = ml_dtypes.bfloat16
    inv_freq = 1.0 / (THETA ** (np.arange(0, HD, 2, dtype=np.float64) / HD))

    # multiplicative 0/1 causal masks for the 4 diagonal-block phases,
    # pre-reshaped to the [128, 4*512] SBUF row layout
    maskb = np.zeros((4, 128, 512), dtype=bf)
    jl = np.arange(128)[:, None]
    il = np.arange(512)[None, :]
    for p in range(4):
        maskb[p] = (128 * p + jl <= il).astype(bf)
    maskb = np.ascontiguousarray(maskb.transpose(1, 0, 2).reshape(128, 4 * 512))

    in_maps = []
    for d in range(NCORES):
        b, g = d // 4, d % 4
        def to_sbuf_rows(a, wdim):
            # [K_total, wdim] -> [128, KT * wdim]: chunk kk lands at
            # columns [kk*wdim, (kk+1)*wdim) with hidden-dim on partitions.
            c = a.astype(bf).reshape(KT, 128, wdim)
            return np.ascontiguousarray(c.transpose(1, 0, 2).reshape(128, KT * wdim))

        hsT = np.ascontiguousarray(hidden_states[b].T).astype(bf).reshape(KT, 128, S)
        wqT = to_sbuf_rows(Wq[g * QDIM:(g + 1) * QDIM].T, QDIM)
        wkT = to_sbuf_rows(Wk[g * HD:(g + 1) * HD].T, HD)
        wvT = to_sbuf_rows(Wv[g * HD:(g + 1) * HD].T, HD)
        woT = np.ascontiguousarray(
            Wo[:, g * QDIM:(g + 1) * QDIM].T.astype(bf).reshape(4, 128, HID)
            .transpose(1, 0, 2).reshape(128, 4 * HID))
        freqs = position_ids[b].astype(np.float64)[:, None] * inv_freq[None, :]  # [S, 64]
        emb = np.concatenate([freqs, freqs], axis=1)  # [S, 128]
        cosT = np.cos(emb).T.astype(bf)
        sinT = np.sin(emb).T.astype(bf)
        in_maps.append({
            "hsT": hsT, "wqT": wqT, "wkT": wkT, "wvT": wvT, "woT": woT,
            "cosT": np.ascontiguousarray(cosT),
            "sinT": np.ascontiguousarray(sinT),
            "maskb": maskb, "identD": np.eye(128, dtype=bf),
        })
    return in_maps


def kernel(hidden_states, Wq, Wk, Wv, Wo, position_ids, _trace=False, _tmpdir=None):
    from concourse.bass_utils import run_bass_kernel_spmd

    if "nc" not in _CACHE:
        _CACHE["nc"] = _build_nc()
    nc = _CACHE["nc"]

    in_maps = _host_prep(
        np.asarray(hidden_states), np.asarray(Wq), np.asarray(Wk),
        np.asarray(Wv), np.asarray(Wo), np.asarray(position_ids),
    )
    res = run_bass_kernel_spmd(
        nc, in_maps, core_ids=list(range(NCORES)), trace=_trace, tmpdir=_tmpdir
    )
    _CACHE["last_result"] = res

    out = np.zeros((B, S, NH * HD), dtype=np.float32)
    for d in range(NCORES):
        out[d // 4] += np.asarray(res.results[d]["out"], dtype=np.float32)
    return out
